# revision 2
# baseline (speedup 1.0000x reference)
"""Bahdanau-attention scoring kernel for Trainium2 (8 NeuronCores, SPMD).

Computes softmax_s( v . tanh(hidden @ Wh^T + enc @ We^T + b) ) for
hidden [32,1024], enc [32,2048,1024]  ->  out [32,2048].

fp8 main GEMM + exact top-k refine.

Main pass runs the enc @ We^T GEMM in fp8e4 with DoubleRow perf mode
(K=256 per matmul instruction, ~1.5-1.8x the bf16 streaming rate).  W is
pre-scaled x256 and enc x16 on the host so both sit in e4m3's normal
range; the 2^-12 dequant rides the tanh activation's scale.  fp8 alone
gives score errors ~0.2 -- far too big for the softmax top -- so the few
entries that matter per row are recomputed exactly: scores land in a
[16,512] layout (partition = 4*row + 512-block) where one DVE max8 +
max_index picks the top-8 of every 512-block (32 candidates/row; the
data shows at most 13 entries within 3.5 of a row max, so coverage
holds with large margin).  A gpsimd dma_gather(transpose=True) pulls
the 128 selected enc rows from DRAM straight into [128h, 8k, 128idx]
layout, a small fp16 GEMM (with the fp32-exact q bias folded in as two
fp16 hi/lo contraction planes against a static one-hot b-mask)
recomputes those scores exactly, and the softmax denominator is patched
with sum(exp(s_exact) - exp(s_approx)).  The device ships the fp8 prob
map plus 32 refined (index, prob) pairs per row; the host splices them
during unshard.  Simulated end-to-end rel err: 1.9e-3 (gate 2e-2).
"""

from contextlib import ExitStack

import ml_dtypes
import numpy as np

import concourse.bacc as bacc
import concourse.bass_isa as bass_isa
import concourse.library_config as library_config
import concourse.mybir as mybir
import concourse.tile as tile
from concourse.bass_utils import run_bass_kernel_spmd

HID = 1024
BATCH = 32
SRC = 2048
NCORES = 8
BLOC = BATCH // NCORES  # 4 batch rows per core
KT = HID // 128  # 8 k-tiles over the contraction dim
MT = HID // 128  # 8 m-tiles over the output-feature dim
NCHUNK = 512  # matmul moving free dim / psum bank width (fp32 out)
SCHUNKS = SRC // NCHUNK  # 4 s-chunks per batch row
NCHUNKS = BLOC * SCHUNKS  # 16 chunks per core
NSEL = 8  # refine candidates per 512-block (max8)
NIDX = 16 * NSEL  # 128 gathered rows per core
SW = 256.0  # host scale on We^T before e4m3 cast
SE = 16.0  # host scale on enc before e4m3 cast
DEQ = 1.0 / (SW * SE)

F32 = mybir.dt.float32
F16 = mybir.dt.float16
F8 = mybir.dt.float8e4
U16 = mybir.dt.uint16
I16 = mybir.dt.int16
DR = mybir.MatmulPerfMode.DoubleRow

_compiled = {}
_last_results = None


def _build_kernel(ctx: ExitStack, tc: tile.TileContext, aps: dict):
    nc = tc.nc
    enc8_d = aps["enc8_t"]  # [BLOC, KT, 128, SRC] fp8 (b, k, p, s), enc*16
    enc16_d = aps["enc16_t"]  # [BLOC*SRC, HID] f16 row-major (gather source)
    w8_d = aps["w8_t"]  # [128, KT, MT*128] fp8 (p, k, o), We^T*256
    w16_d = aps["w16_t"]  # [MT, 128, KT*128] f16 (m, p, k*o')
    q_d = aps["q_t"]  # [128, MT*BLOC] f32 (Wh@hid^T + b, host)
    qhl_d = aps["qhl_t"]  # [8, MT*128] f16 (rows 0-3 q_hi, 4-7 q_lo)
    mask_d = aps["mask_t"]  # [8, NIDX] f16 one-hot b-mask for gathered cols
    blkoff_d = aps["blkoff_t"]  # [128, 1] u16 = 512*(p%16)
    outp_d = aps["out_p"]  # [BLOC, SRC] f32 (fp8 probs)
    outs_d = aps["out_sel"]  # [16, 8] f32 (exp of refined scores)
    outz_d = aps["out_z"]  # [BLOC, 1] f32 (patched denominators)
    outi_d = aps["out_idx"]  # [16, 8] u16 (core-global row index)

    w_pool = ctx.enter_context(tc.tile_pool(name="w", bufs=1))
    small_pool = ctx.enter_context(tc.tile_pool(name="small", bufs=1))
    enc_pool = ctx.enter_context(tc.tile_pool(name="enc", bufs=4))
    tanh_pool = ctx.enter_context(tc.tile_pool(name="tanh", bufs=16))
    score_pool = ctx.enter_context(tc.tile_pool(name="score", bufs=1))
    thr_pool = ctx.enter_context(tc.tile_pool(name="thr", bufs=8))
    srow_pool = ctx.enter_context(tc.tile_pool(name="srow", bufs=2))
    psum_e = ctx.enter_context(tc.tile_pool(name="psum_e", bufs=6, space="PSUM"))
    psum_s = ctx.enter_context(tc.tile_pool(name="psum_s", bufs=2, space="PSUM"))

    nc.gpsimd.load_library(library_config.mlp)

    # force the ACT table load (tanh/exp set) while DMAs are in flight
    warm = small_pool.tile([1, 2], F32)
    nc.vector.memset(warm[:], 0.0)
    nc.scalar.activation(
        warm[0:1, 1:2], warm[0:1, 0:1], mybir.ActivationFunctionType.Tanh
    )

    # scalar-queue DMAs: small resident tensors
    q_sb = small_pool.tile([128, MT * BLOC], F32)
    nc.scalar.dma_start(out=q_sb[:], in_=q_d[:])
    v8_sb = small_pool.tile([128, KT, 128], F8)
    nc.scalar.dma_start(out=v8_sb[:], in_=aps["v8_t"][:])
    v16_sb = small_pool.tile([128, MT], F16)
    nc.scalar.dma_start(out=v16_sb[:], in_=aps["v16_t"][:])
    qhl_sb = small_pool.tile([8, MT * 128], F16)
    nc.scalar.dma_start(out=qhl_sb[:], in_=qhl_d[:])
    mask_sb = small_pool.tile([8, NIDX], F16)
    nc.scalar.dma_start(out=mask_sb[:], in_=mask_d[:])
    blkoff_sb = small_pool.tile([128, 1], U16)
    nc.scalar.dma_start(out=blkoff_sb[:], in_=blkoff_d[:])
    bmask_sb = small_pool.tile([BLOC, NIDX], F32)
    nc.scalar.dma_start(out=bmask_sb[:], in_=aps["bmask_t"][:])

    # sync-queue: first enc chunk halves + fp8 weights (per-m so m0 can
    # start early), then w16 for the refine pass rides behind chunk 1
    enc0_sb = enc_pool.tile([128, KT, NCHUNK], F8, tag="enc", name="enc0_sb")
    w8_sb = w_pool.tile([128, KT, MT * 128], F8)
    nc.sync.dma_start(
        out=enc0_sb[:, 0:4, :],
        in_=enc8_d[0].rearrange("k p s -> p k s")[:, 0:4, 0:NCHUNK],
    )
    nc.sync.dma_start(out=w8_sb[:, :, 0:256], in_=w8_d[:, :, 0:256])
    nc.sync.dma_start(
        out=enc0_sb[:, 4:8, :],
        in_=enc8_d[0].rearrange("k p s -> p k s")[:, 4:8, 0:NCHUNK],
    )
    for m in range(2, MT, 2):
        nc.sync.dma_start(
            out=w8_sb[:, :, m * 128 : (m + 2) * 128],
            in_=w8_d[:, :, m * 128 : (m + 2) * 128],
        )

    # warm the PE clock (HAM) with dummy matmuls while DMAs land
    scr = small_pool.tile([128, NCHUNK], F16)
    nc.vector.memset(scr[:], 0.0)
    for _ in range(14):
        wp = psum_e.tile([128, NCHUNK], F32, tag="ep", name="warmmm")
        nc.tensor.matmul(
            wp[:], lhsT=scr[:, 0:128], rhs=scr[:], start=True, stop=True
        )

    scr3_d = aps["scr3_t"]  # [16, NSEL] u16 DRAM scratch
    # score/selection state
    scores16 = score_pool.tile([16, NCHUNK], F32, name="scores16")
    scores4 = score_pool.tile([BLOC, SRC], F32, name="scores4")
    expS = score_pool.tile([BLOC, SRC], F32, name="expS")
    esumq = score_pool.tile([BLOC, SCHUNKS], F32, name="esumq")
    esum4 = score_pool.tile([BLOC, 1], F32, name="esum4")
    vals16 = score_pool.tile([16, NSEL], F32, name="vals16")
    sg16 = score_pool.tile([16, NSEL], U16, name="sg16")
    sg128 = score_pool.tile([128, NSEL], U16, name="sg128")

    def energy_m(enc_sb, m, kk0, kk1):
        ep = psum_e.tile([128, NCHUNK], F32, tag="ep", name="ep")
        for kk in range(kk0, kk1):
            nc.tensor.matmul(
                ep[:],
                lhsT=w8_sb[:, 2 * kk : 2 * kk + 2, m * 128 : (m + 1) * 128],
                rhs=enc_sb[:, 2 * kk : 2 * kk + 2, :],
                start=(kk == 0),
                stop=(kk == KT // 2 - 1),
                perf_mode=DR,
            )
        return ep

    def energy_part(ep, enc_sb, m, kk0, kk1):
        for kk in range(kk0, kk1):
            nc.tensor.matmul(
                ep[:],
                lhsT=w8_sb[:, 2 * kk : 2 * kk + 2, m * 128 : (m + 1) * 128],
                rhs=enc_sb[:, 2 * kk : 2 * kk + 2, :],
                start=(kk == 0),
                stop=(kk == KT // 2 - 1),
                perf_mode=DR,
            )

    def tanh_m(th8, b, m, ep):
        # tanh -> fp8 into half of a pair tile (PE v-dot reads pairs)
        nc.scalar.activation(
            th8[:, m % 2, :],
            ep[:],
            mybir.ActivationFunctionType.Tanh,
            bias=q_sb[:, m * BLOC + b : m * BLOC + b + 1],
            scale=DEQ,
        )

    def vdot_pe(th8_tiles):
        # score row via 4 DoubleRow matmuls; v sits in column 0 of a full
        # 128-wide stationary tile (DoubleRow needs >=16B between planes),
        # so row 0 of the psum carries the score and rows 1-127 are zero
        vp = psum_s.tile([128, NCHUNK], F32, tag="vs", name="vp")
        for j in range(4):
            nc.tensor.matmul(
                vp[:],
                lhsT=v8_sb[:, 2 * j : 2 * j + 2, :],
                rhs=th8_tiles[j][:],
                start=(j == 0),
                stop=(j == 3),
                perf_mode=DR,
            )
        return vp

    def flush_scores(vp, b, s):
        # DMA can't read PSUM: stage the score row through SBUF on DVE
        row = srow_pool.tile([1, NCHUNK], F32, name="srow")
        nc.vector.tensor_scalar_add(row[:], vp[0:1, :], 0.0)
        p16 = b * SCHUNKS + s
        nc.sync.dma_start(out=scores16[p16 : p16 + 1, :], in_=row[:])
        nc.sync.dma_start(
            out=scores4[b : b + 1, s * NCHUNK : (s + 1) * NCHUNK],
            in_=row[:],
        )
        if b == BLOC - 1:
            # all 4 rows of column-block s are final: exp this column strip
            # (hidden under the remaining chunks' PE work)
            nc.scalar.activation(
                expS[:, s * NCHUNK : (s + 1) * NCHUNK],
                scores4[:, s * NCHUNK : (s + 1) * NCHUNK],
                mybir.ActivationFunctionType.Exp,
                accum_out=esumq[:, s : s + 1],
            )

    # --- main loop: 16 chunks of 512 s-values ---------------------------
    # chunk i's v-dot matmuls are deferred into chunk i+1's PE stream
    # (after its 6th m-group) so the PE never waits on ScalarE tanh
    pend = None  # (th8_tiles, b, s) of chunk i-1
    for i in range(NCHUNKS):
        b, s = divmod(i, SCHUNKS)
        if i == 0:
            enc_sb = enc0_sb
        else:
            enc_sb = enc_pool.tile([128, KT, NCHUNK], F8, tag="enc")
            nc.sync.dma_start(
                out=enc_sb[:],
                in_=enc8_d[b].rearrange("k p s -> p k s")[
                    :, :, s * NCHUNK : (s + 1) * NCHUNK
                ],
            )
        if 4 <= i < 4 + MT:
            # refine weights: 2 MiB staggered per-m behind enc prefetch
            if i == 4:
                w16_sb = w_pool.tile([128, MT, KT * 128], F16, name="w16_sb")
            nc.sync.dma_start(out=w16_sb[:, i - 4, :], in_=w16_d[i - 4])
        th8_tiles = [
            tanh_pool.tile([128, 2, NCHUNK], F8, name="th8") for _ in range(4)
        ]
        m_start = 0
        if i == 0:
            # chunk 0: split-k for the first two m-groups so compute
            # starts once half of enc0 (+ first w8 block) has landed
            m_start = 2
            ep0 = psum_e.tile([128, NCHUNK], F32, tag="ep", name="ep")
            ep1 = psum_e.tile([128, NCHUNK], F32, tag="ep", name="ep")
            energy_part(ep0, enc_sb, 0, 0, 2)
            energy_part(ep1, enc_sb, 1, 0, 2)
            energy_part(ep0, enc_sb, 0, 2, KT // 2)
            tanh_m(th8_tiles[0], b, 0, ep0)
            energy_part(ep1, enc_sb, 1, 2, KT // 2)
            tanh_m(th8_tiles[0], b, 1, ep1)
        for m in range(m_start, MT):
            ep = energy_m(enc_sb, m, 0, KT // 2)
            if m == 6 and pend is not None:
                # deferred v-dot of chunk i-1 (its tanhs are long done)
                vp = vdot_pe(pend[0])
                flush_scores(vp, pend[1], pend[2])
                pend = None
            tanh_m(th8_tiles[m // 2], b, m, ep)
        pend = (th8_tiles, b, s)
        if i == NCHUNKS - 1:
            # speculative selection pass A: blocks 0..14 are final (only
            # chunk 15 pending) -- run the full selection and ship the
            # replicated index tile now; row 15's replicas get patched
            # after the last chunk.  Hides the DRAM round trip.
            nc.vector.max(vals16[:], scores16[:])
            idxA = score_pool.tile([16, NSEL], U16, name="idxA")
            nc.vector.max_index(idxA[:], vals16[:], scores16[:])
            nc.vector.tensor_scalar(
                sg16[:],
                idxA[:],
                blkoff_sb[0:16, 0:1],
                None,
                op0=mybir.AluOpType.bitwise_or,
            )
            nc.sync.dma_start(out=scr3_d, in_=sg16[:])
    # last chunk: v-dot immediately (PE waits on its final tanhs briefly)
    vp = vdot_pe(pend[0])
    flush_scores(vp, pend[1], pend[2])

    # --- selection pass B: final block-15 row ---------------------------
    nc.vector.max(vals16[:], scores16[:])
    idx16 = score_pool.tile([16, NSEL], U16, name="idx16")
    nc.vector.max_index(idx16[:], vals16[:], scores16[:])
    nc.vector.tensor_scalar(
        sg16[:],
        idx16[:],
        blkoff_sb[0:16, 0:1],
        None,
        op0=mybir.AluOpType.bitwise_or,
    )
    nc.sync.dma_start(out=outi_d[:], in_=sg16[:])
    # only block 15 changed vs pass A: patch its row in DRAM, then one
    # stride-0 broadcast read rebuilds all 8 replicas
    nc.sync.dma_start(out=scr3_d[15:16, :], in_=sg16[15:16, :])
    nc.sync.dma_start(
        out=sg128[:], in_=scr3_d.unsqueeze(0).broadcast_to((8, 16, NSEL))
    )
    # total exp-sum per row from the 4 column-strip accumulators
    nc.vector.tensor_reduce(
        esum4[:], esumq[:], axis=mybir.AxisListType.X, op=mybir.AluOpType.add
    )
    # selected approx scores to a [1,128] row in gather-column order
    # (via DRAM; latency hidden behind the gather + refine GEMM)
    valsd_d = aps["valsd_t"]  # [16, NSEL] f32 DRAM scratch
    nc.sync.dma_start(out=valsd_d, in_=vals16[:])
    vals_row = score_pool.tile([1, NIDX], F32, name="vals_row")
    nc.sync.dma_start(out=vals_row[:], in_=valsd_d.rearrange("p r -> r p"))
    expold_row = score_pool.tile([1, NIDX], F32, name="expold_row")
    nc.scalar.activation(
        expold_row[:], vals_row[:], mybir.ActivationFunctionType.Exp
    )

    # --- refine pass ----------------------------------------------------
    g_sb = score_pool.tile([128, KT, NIDX], F16, name="g_sb")
    nc.gpsimd.dma_gather(
        g_sb[:],
        enc16_d[:],
        sg128[:].bitcast(I16),
        num_idxs=NIDX,
        num_idxs_reg=NIDX,
        elem_size=HID,
        transpose=True,
    )

    # dummy matmuls during the selection/gather wait keep the PE clock
    # (HAM) warm so the refine GEMM runs at 2.4 GHz; the two leading
    # q-plane matmuls also slot in here (they don't need gathered data)
    epr01 = []
    for m in range(2):
        epr = psum_e.tile([128, NCHUNK], F32, tag="ep", name="epr")
        nc.tensor.matmul(
            epr[:, 0:NIDX],
            lhsT=qhl_sb[:, m * 128 : (m + 1) * 128],
            rhs=mask_sb[:],
            start=True,
            stop=False,
        )
        epr01.append(epr)
    wp2 = psum_s.tile([128, NCHUNK], F32, tag="vs", name="warm2")
    for _ in range(150):
        nc.tensor.matmul(
            wp2[:, 0:256],
            lhsT=scr[:, 0:128],
            rhs=scr[:, 0:256],
            start=True,
            stop=True,
        )
    vpr = psum_s.tile([1, NCHUNK], F32, tag="vs", name="vpr")
    thr_tiles = []
    for m in range(MT):
        if m < 2:
            epr = epr01[m]
            for k in range(KT):
                nc.tensor.matmul(
                    epr[:, 0:NIDX],
                    lhsT=w16_sb[:, m, k * 128 : (k + 1) * 128],
                    rhs=g_sb[:, k, :],
                    start=False,
                    stop=(k == KT - 1),
                )
        else:
            epr = psum_e.tile([128, NCHUNK], F32, tag="ep", name="epr")
            for k in range(KT):
                nc.tensor.matmul(
                    epr[:, 0:NIDX],
                    lhsT=w16_sb[:, m, k * 128 : (k + 1) * 128],
                    rhs=g_sb[:, k, :],
                    start=(k == 0),
                    stop=False,
                )
            nc.tensor.matmul(
                epr[:, 0:NIDX],
                lhsT=qhl_sb[:, m * 128 : (m + 1) * 128],
                rhs=mask_sb[:],
                start=False,
                stop=True,
            )
        if m >= 1:
            # deferred exact v-dot of group m-1 (its tanh is done)
            nc.tensor.matmul(
                vpr[0:1, 0:NIDX],
                lhsT=v16_sb[:, m - 1 : m],
                rhs=thr_tiles[m - 1][:],
                start=(m == 1),
                stop=False,
            )
        thr = thr_pool.tile([128, NIDX], F16, name="thr")
        nc.scalar.activation(
            thr[:],
            epr[:, 0:NIDX],
            mybir.ActivationFunctionType.Tanh,
            scale=1.0,
        )
        thr_tiles.append(thr)
    nc.tensor.matmul(
        vpr[0:1, 0:NIDX],
        lhsT=v16_sb[:, MT - 1 : MT],
        rhs=thr_tiles[MT - 1][:],
        start=False,
        stop=True,
    )

    # exp + delta stay in the [1,128] row domain; the per-row delta sums
    # come from a partition-broadcast read masked by a host one-hot, so no
    # DRAM hops sit on the critical path
    pk = score_pool.tile([1, 2 * NIDX], F32, name="pk")
    nc.scalar.activation(
        pk[:, 0:NIDX], vpr[0:1, 0:NIDX], mybir.ActivationFunctionType.Exp
    )
    nc.vector.scalar_tensor_tensor(
        pk[:, NIDX : 2 * NIDX],
        pk[:, 0:NIDX],
        1.0,
        expold_row[:],
        op0=mybir.AluOpType.mult,
        op1=mybir.AluOpType.subtract,
    )
    bc4 = score_pool.tile([BLOC, NIDX], F32, name="bc4")
    nc.gpsimd.partition_broadcast(
        bc4[:], pk[0:1, NIDX : 2 * NIDX], channels=BLOC
    )
    dzq4 = score_pool.tile([BLOC, NIDX], F32, name="dzq4")
    dz4 = score_pool.tile([BLOC, 1], F32, name="dz4")
    nc.vector.scalar_tensor_tensor(
        dzq4[:],
        bc4[:],
        1.0,
        bmask_sb[:],
        op0=mybir.AluOpType.mult,
        op1=mybir.AluOpType.mult,
        accum_out=dz4[:, 0:1],
    )
    z4 = score_pool.tile([BLOC, 1], F32, name="z4")
    nc.vector.scalar_tensor_tensor(
        z4[:],
        esum4[:],
        1.0,
        dz4[:],
        op0=mybir.AluOpType.mult,
        op1=mybir.AluOpType.add,
    )
    rcp4 = score_pool.tile([BLOC, 1], F32, name="rcp4")
    nc.vector.reciprocal(rcp4[:], z4[:])
    probs4 = score_pool.tile([BLOC, SRC], F32, name="probs4")
    half = SRC // 2
    nc.vector.tensor_scalar_mul(
        probs4[:, 0:half], expS[:, 0:half], rcp4[:, 0:1]
    )
    nc.sync.dma_start(out=outp_d[:, 0:half], in_=probs4[:, 0:half])
    nc.vector.tensor_scalar_mul(
        probs4[:, half:], expS[:, half:], rcp4[:, 0:1]
    )
    nc.sync.dma_start(out=outp_d[:, half:], in_=probs4[:, half:])
    nc.sync.dma_start(out=outs_d[:], in_=pk[:, 0:NIDX])
    nc.sync.dma_start(out=outz_d[:], in_=z4[:])
    if "dbg_scores" in aps:
        nc.sync.dma_start(out=aps["dbg_scores"][:], in_=scores4[:])
        nc.sync.dma_start(out=aps["dbg_esum"][:], in_=esum4[:])
        nc.sync.dma_start(out=aps["dbg_dzq"][:], in_=dzq[:])
        nc.sync.dma_start(out=aps["dbg_g"][:], in_=g_sb[:])


def build_nc(dbg=False):
    nc = bacc.Bacc("TRN2", target_bir_lowering=False, debug=False)
    aps = {
        "enc8_t": nc.dram_tensor(
            "enc8_t", [BLOC, KT, 128, SRC], F8, kind="ExternalInput"
        ).ap(),
        "enc16_t": nc.dram_tensor(
            "enc16_t", [BLOC * SRC, HID], F16, kind="ExternalInput"
        ).ap(),
        "w8_t": nc.dram_tensor(
            "w8_t", [128, KT, MT * 128], F8, kind="ExternalInput"
        ).ap(),
        "w16_t": nc.dram_tensor(
            "w16_t", [MT, 128, KT * 128], F16, kind="ExternalInput"
        ).ap(),
        "q_t": nc.dram_tensor(
            "q_t", [128, MT * BLOC], F32, kind="ExternalInput"
        ).ap(),
        "qhl_t": nc.dram_tensor(
            "qhl_t", [8, MT * 128], F16, kind="ExternalInput"
        ).ap(),
        "mask_t": nc.dram_tensor(
            "mask_t", [8, NIDX], F16, kind="ExternalInput"
        ).ap(),
        "blkoff_t": nc.dram_tensor(
            "blkoff_t", [128, 1], U16, kind="ExternalInput"
        ).ap(),
        "bmask_t": nc.dram_tensor(
            "bmask_t", [BLOC, NIDX], F32, kind="ExternalInput"
        ).ap(),
        "v8_t": nc.dram_tensor(
            "v8_t", [128, KT, 128], F8, kind="ExternalInput"
        ).ap(),
        "v16_t": nc.dram_tensor(
            "v16_t", [128, MT], F16, kind="ExternalInput"
        ).ap(),
        "out_p": nc.dram_tensor(
            "out_p", [BLOC, SRC], F32, kind="ExternalOutput"
        ).ap(),
        "out_sel": nc.dram_tensor(
            "out_sel", [1, NIDX], F32, kind="ExternalOutput"
        ).ap(),
        "out_z": nc.dram_tensor(
            "out_z", [BLOC, 1], F32, kind="ExternalOutput"
        ).ap(),
        "out_idx": nc.dram_tensor(
            "out_idx", [16, NSEL], U16, kind="ExternalOutput"
        ).ap(),
        "scr_t": nc.dram_tensor(
            "scr_t", [16, 2 * NSEL], F32, kind="Internal"
        ).ap(),
        "valsd_t": nc.dram_tensor(
            "valsd_t", [16, NSEL], F32, kind="Internal"
        ).ap(),
        "scr2_t": nc.dram_tensor("scr2_t", [16], F32, kind="Internal").ap(),
        "scr3_t": nc.dram_tensor(
            "scr3_t", [16, NSEL], U16, kind="Internal"
        ).ap(),
    }
    if dbg:
        aps["dbg_scores"] = nc.dram_tensor(
            "dbg_scores", [BLOC, SRC], F32, kind="ExternalOutput"
        ).ap()
        aps["dbg_esum"] = nc.dram_tensor(
            "dbg_esum", [BLOC, 1], F32, kind="ExternalOutput"
        ).ap()
        aps["dbg_dzq"] = nc.dram_tensor(
            "dbg_dzq", [BLOC, 4], F32, kind="ExternalOutput"
        ).ap()
        aps["dbg_selex"] = nc.dram_tensor(
            "dbg_selex", [16, NSEL], F32, kind="ExternalOutput"
        ).ap()
        aps["dbg_expold"] = nc.dram_tensor(
            "dbg_expold", [16, NSEL], F32, kind="ExternalOutput"
        ).ap()
        aps["dbg_redr"] = nc.dram_tensor(
            "dbg_redr", [1, NIDX], F32, kind="ExternalOutput"
        ).ap()
        aps["dbg_g"] = nc.dram_tensor(
            "dbg_g", [128, KT, NIDX], F16, kind="ExternalOutput"
        ).ap()
    with tile.TileContext(nc, pool_alloc_mode="queue") as tc, ExitStack() as ctx:
        _build_kernel(ctx, tc, aps)
    nc.compile()
    return nc


def _prep_shared(hidden, attn_w, attn_b, v):
    w_e_t = np.ascontiguousarray(attn_w[:, HID:].T)  # [h, o]
    # fp8 weights: [h, o] -> [kt, 128, o] -> [128, kt, o], scaled x256
    w8_t = np.ascontiguousarray(
        (w_e_t * SW)
        .astype(ml_dtypes.float8_e4m3)
        .reshape(KT, 128, MT * 128)
        .transpose(1, 0, 2)
    )
    # fp16 refine weights (baseline layout)
    w16_t = np.ascontiguousarray(
        w_e_t.reshape(KT, 128, MT, 128)
        .transpose(2, 1, 0, 3)
        .reshape(MT, 128, KT * 128)
        .astype(np.float16)
    )
    v16_np = v.reshape(MT, 128).T  # [128, MT]
    v8_t = np.zeros((128, KT, 128), dtype=ml_dtypes.float8_e4m3)
    v8_t[:, :, 0] = v16_np.astype(ml_dtypes.float8_e4m3)
    v16_t = np.ascontiguousarray(v16_np.astype(np.float16))
    # q[o, b] = Wh @ hidden^T + b, fp32 on host (tiny GEMM)
    q_all = (hidden @ attn_w[:, :HID].T + attn_b).astype(np.float32)  # [B, H]
    q_cores = []
    qhl_cores = []
    for c in range(NCORES):
        qc = q_all[c * BLOC : (c + 1) * BLOC].T  # [HID, BLOC]
        q_cores.append(
            np.ascontiguousarray(
                qc.reshape(MT, 128, BLOC)
                .transpose(1, 0, 2)
                .reshape(128, MT * BLOC)
                .astype(np.float32)
            )
        )
        qb = q_all[c * BLOC : (c + 1) * BLOC]  # [BLOC, HID]
        qhi = qb.astype(np.float16)
        qlo = (qb - qhi.astype(np.float32)).astype(np.float16)
        qhl_cores.append(np.ascontiguousarray(np.concatenate([qhi, qlo], axis=0)))
    # one-hot mask: gathered column t belongs to b(t) = (t%16)//4
    mask = np.zeros((8, NIDX), dtype=np.float16)
    t = np.arange(NIDX)
    bt = (t % 16) // 4
    for j in range(4):
        mask[j, bt == j] = 1.0
        mask[j + 4, bt == j] = 1.0
    blkoff = ((np.arange(128, dtype=np.uint16) % 16) * 512).reshape(128, 1)
    t = np.arange(NIDX)
    bmask = ((t % 16) // 4 == np.arange(BLOC)[:, None]).astype(np.float32)
    return w8_t, w16_t, v8_t, v16_t, q_cores, qhl_cores, mask, blkoff, bmask


def kernel(hidden, encoder_outputs, attn_w, attn_b, v):
    global _last_results
    hidden = np.asarray(hidden, dtype=np.float32)
    encoder_outputs = np.asarray(encoder_outputs, dtype=np.float32)
    attn_w = np.asarray(attn_w, dtype=np.float32)
    attn_b = np.asarray(attn_b, dtype=np.float32)
    v = np.asarray(v, dtype=np.float32)

    if "nc" not in _compiled:
        _compiled["nc"] = build_nc()
    nc = _compiled["nc"]

    w8_t, w16_t, v8_t, v16_t, q_cores, qhl_cores, mask, blkoff, bmask = _prep_shared(
        hidden, attn_w, attn_b, v
    )
    in_maps = []
    for c in range(NCORES):
        enc_c = encoder_outputs[c * BLOC : (c + 1) * BLOC]  # [bloc, s, h]
        enc8_t = np.ascontiguousarray(
            (np.ascontiguousarray(enc_c.transpose(0, 2, 1)) * SE)
            .astype(ml_dtypes.float8_e4m3)
            .reshape(BLOC, KT, 128, SRC)
        )
        enc16_t = np.ascontiguousarray(
            enc_c.reshape(BLOC * SRC, HID).astype(np.float16)
        )
        in_maps.append(
            {
                "enc8_t": enc8_t,
                "enc16_t": enc16_t,
                "w8_t": w8_t,
                "w16_t": w16_t,
                "q_t": q_cores[c],
                "qhl_t": qhl_cores[c],
                "mask_t": mask,
                "blkoff_t": blkoff,
                "bmask_t": bmask,
                "v8_t": v8_t,
                "v16_t": v16_t,
            }
        )

    res = run_bass_kernel_spmd(nc, in_maps, list(range(NCORES)))
    _last_results = res
    out = np.empty((BATCH, SRC), dtype=np.float32)
    for c in range(NCORES):
        r = res.results[c]
        probs = np.asarray(r["out_p"], dtype=np.float32).copy()  # [BLOC, SRC]
        expsel = np.asarray(r["out_sel"], dtype=np.float32).ravel()  # [NIDX]
        z = np.asarray(r["out_z"], dtype=np.float32).reshape(BLOC)  # [BLOC]
        idxg = np.asarray(r["out_idx"]).astype(np.int64)  # [16, 8]
        t = np.arange(NIDX)
        idxr = idxg[t % 16, t // 16]  # gather-column order
        b_of = idxr // SRC
        pos = idxr % SRC
        probs[b_of, pos] = expsel / z[b_of]
        out[c * BLOC : (c + 1) * BLOC] = probs
    return out


# revision 3
# speedup vs baseline: 1.0075x; 1.0075x over previous
"""Bahdanau-attention scoring kernel for Trainium2 (8 NeuronCores, SPMD).

Computes softmax_s( v . tanh(hidden @ Wh^T + enc @ We^T + b) ) for
hidden [32,1024], enc [32,2048,1024]  ->  out [32,2048].

v2: fp8 main GEMM + exact top-k refine.

Main pass runs the enc @ We^T GEMM in fp8e4 with DoubleRow perf mode
(K=256 per matmul instruction, ~1.5-1.8x the bf16 streaming rate).  W is
pre-scaled x256 and enc x16 on the host so both sit in e4m3's normal
range; the 2^-12 dequant rides the tanh activation's scale.  fp8 alone
gives score errors ~0.2 -- far too big for the softmax top -- so the few
entries that matter per row are recomputed exactly: scores land in a
[16,512] layout (partition = 4*row + 512-block) where one DVE max8 +
max_index picks the top-8 of every 512-block (32 candidates/row; the
data shows at most 13 entries within 3.5 of a row max, so coverage
holds with large margin).  A gpsimd dma_gather(transpose=True) pulls
the 128 selected enc rows from DRAM straight into [128h, 8k, 128idx]
layout, a small fp16 GEMM (with the fp32-exact q bias folded in as two
fp16 hi/lo contraction planes against a static one-hot b-mask)
recomputes those scores exactly, and the softmax denominator is patched
with sum(exp(s_exact) - exp(s_approx)).  The device ships the fp8 prob
map plus 32 refined (index, prob) pairs per row; the host splices them
during unshard.  Simulated end-to-end rel err: 1.9e-3 (gate 2e-2).
"""

from contextlib import ExitStack

import ml_dtypes
import numpy as np

import concourse.bacc as bacc
import concourse.bass_isa as bass_isa
import concourse.library_config as library_config
import concourse.mybir as mybir
import concourse.tile as tile
from concourse.bass_utils import run_bass_kernel_spmd

HID = 1024
BATCH = 32
SRC = 2048
NCORES = 8
BLOC = BATCH // NCORES  # 4 batch rows per core
KT = HID // 128  # 8 k-tiles over the contraction dim
MT = HID // 128  # 8 m-tiles over the output-feature dim
NCHUNK = 512  # matmul moving free dim / psum bank width (fp32 out)
SCHUNKS = SRC // NCHUNK  # 4 s-chunks per batch row
NCHUNKS = BLOC * SCHUNKS  # 16 chunks per core
NSEL = 8  # refine candidates per 512-block (max8)
NIDX = 16 * NSEL  # 128 gathered rows per core
SW = 256.0  # host scale on We^T before e4m3 cast
SE = 16.0  # host scale on enc before e4m3 cast
DEQ = 1.0 / (SW * SE)

F32 = mybir.dt.float32
F16 = mybir.dt.float16
F8 = mybir.dt.float8e4
U16 = mybir.dt.uint16
I16 = mybir.dt.int16
DR = mybir.MatmulPerfMode.DoubleRow

_compiled = {}
_last_results = None


def _build_kernel(ctx: ExitStack, tc: tile.TileContext, aps: dict):
    nc = tc.nc
    enc8_d = aps["enc8_t"]  # [BLOC, KT, 128, SRC] fp8 (b, k, p, s), enc*16
    enc16_d = aps["enc16_t"]  # [BLOC*SRC, HID] f16 row-major (gather source)
    w8_d = aps["w8_t"]  # [128, KT, MT*128] fp8 (p, k, o), We^T*256
    w16_d = aps["w16_t"]  # [MT, 128, KT*128] f16 (m, p, k*o')
    q_d = aps["q_t"]  # [128, MT*BLOC] f32 (Wh@hid^T + b, host)
    qhl_d = aps["qhl_t"]  # [8, MT*128] f16 (rows 0-3 q_hi, 4-7 q_lo)
    mask_d = aps["mask_t"]  # [8, NIDX] f16 one-hot b-mask for gathered cols
    blkoff_d = aps["blkoff_t"]  # [128, 1] u16 = 512*(p%16)
    outp_d = aps["out_p"]  # [BLOC, SRC] f32 (fp8 probs)
    outs_d = aps["out_sel"]  # [16, 8] f32 (exp of refined scores)
    outz_d = aps["out_z"]  # [BLOC, 1] f32 (patched denominators)
    outi_d = aps["out_idx"]  # [16, 8] u16 (core-global row index)

    w_pool = ctx.enter_context(tc.tile_pool(name="w", bufs=1))
    small_pool = ctx.enter_context(tc.tile_pool(name="small", bufs=1))
    enc_pool = ctx.enter_context(tc.tile_pool(name="enc", bufs=4))
    tanh_pool = ctx.enter_context(tc.tile_pool(name="tanh", bufs=16))
    score_pool = ctx.enter_context(tc.tile_pool(name="score", bufs=1))
    thr_pool = ctx.enter_context(tc.tile_pool(name="thr", bufs=8))
    srow_pool = ctx.enter_context(tc.tile_pool(name="srow", bufs=2))
    psum_e = ctx.enter_context(tc.tile_pool(name="psum_e", bufs=6, space="PSUM"))
    psum_s = ctx.enter_context(tc.tile_pool(name="psum_s", bufs=2, space="PSUM"))

    nc.gpsimd.load_library(library_config.mlp)

    # force the ACT table load (tanh/exp set) while DMAs are in flight
    warm = small_pool.tile([1, 2], F32)
    nc.vector.memset(warm[:], 0.0)
    nc.scalar.activation(
        warm[0:1, 1:2], warm[0:1, 0:1], mybir.ActivationFunctionType.Tanh
    )

    # scalar-queue DMAs: small resident tensors
    q_sb = small_pool.tile([128, MT * BLOC], F32)
    nc.scalar.dma_start(out=q_sb[:], in_=q_d[:])
    v8_sb = small_pool.tile([128, KT, 128], F8)
    nc.scalar.dma_start(out=v8_sb[:], in_=aps["v8_t"][:])
    v16_sb = small_pool.tile([128, MT], F16)
    nc.scalar.dma_start(out=v16_sb[:], in_=aps["v16_t"][:])
    qhl_sb = small_pool.tile([8, MT * 128], F16)
    nc.scalar.dma_start(out=qhl_sb[:], in_=qhl_d[:])
    mask_sb = small_pool.tile([8, NIDX], F16)
    nc.scalar.dma_start(out=mask_sb[:], in_=mask_d[:])
    blkoff_sb = small_pool.tile([128, 1], U16)
    nc.scalar.dma_start(out=blkoff_sb[:], in_=blkoff_d[:])
    bmask_sb = small_pool.tile([BLOC, NIDX], F32)
    nc.scalar.dma_start(out=bmask_sb[:], in_=aps["bmask_t"][:])

    # sync-queue: first enc chunk halves + fp8 weights (per-m so m0 can
    # start early), then w16 for the refine pass rides behind chunk 1
    enc0_sb = enc_pool.tile([128, KT, NCHUNK], F8, tag="enc", name="enc0_sb")
    w8_sb = w_pool.tile([128, KT, MT * 128], F8)
    nc.sync.dma_start(
        out=enc0_sb[:, 0:4, :],
        in_=enc8_d[0].rearrange("k p s -> p k s")[:, 0:4, 0:NCHUNK],
    )
    nc.sync.dma_start(out=w8_sb[:, :, 0:256], in_=w8_d[:, :, 0:256])
    nc.sync.dma_start(
        out=enc0_sb[:, 4:8, :],
        in_=enc8_d[0].rearrange("k p s -> p k s")[:, 4:8, 0:NCHUNK],
    )
    for m in range(2, MT, 2):
        nc.sync.dma_start(
            out=w8_sb[:, :, m * 128 : (m + 2) * 128],
            in_=w8_d[:, :, m * 128 : (m + 2) * 128],
        )

    # warm the PE clock (HAM) with dummy matmuls while DMAs land
    scr = small_pool.tile([128, NCHUNK], F16)
    nc.vector.memset(scr[:], 0.0)
    for _ in range(14):
        wp = psum_e.tile([128, NCHUNK], F32, tag="ep", name="warmmm")
        nc.tensor.matmul(
            wp[:], lhsT=scr[:, 0:128], rhs=scr[:], start=True, stop=True
        )

    scr3_d = aps["scr3_t"]  # [16, NSEL] u16 DRAM scratch
    # score/selection state
    scores16 = score_pool.tile([16, NCHUNK], F32, name="scores16")
    scores4 = score_pool.tile([BLOC, SRC], F32, name="scores4")
    expS = score_pool.tile([BLOC, SRC], F32, name="expS")
    esumq = score_pool.tile([BLOC, SCHUNKS], F32, name="esumq")
    esum4 = score_pool.tile([BLOC, 1], F32, name="esum4")
    vals16 = score_pool.tile([16, NSEL], F32, name="vals16")
    sg16 = score_pool.tile([16, NSEL], U16, name="sg16")
    sg128 = score_pool.tile([128, NSEL], U16, name="sg128")

    def energy_m(enc_sb, m, kk0, kk1):
        ep = psum_e.tile([128, NCHUNK], F32, tag="ep", name="ep")
        for kk in range(kk0, kk1):
            nc.tensor.matmul(
                ep[:],
                lhsT=w8_sb[:, 2 * kk : 2 * kk + 2, m * 128 : (m + 1) * 128],
                rhs=enc_sb[:, 2 * kk : 2 * kk + 2, :],
                start=(kk == 0),
                stop=(kk == KT // 2 - 1),
                perf_mode=DR,
            )
        return ep

    def energy_part(ep, enc_sb, m, kk0, kk1):
        for kk in range(kk0, kk1):
            nc.tensor.matmul(
                ep[:],
                lhsT=w8_sb[:, 2 * kk : 2 * kk + 2, m * 128 : (m + 1) * 128],
                rhs=enc_sb[:, 2 * kk : 2 * kk + 2, :],
                start=(kk == 0),
                stop=(kk == KT // 2 - 1),
                perf_mode=DR,
            )

    def tanh_m(th8, b, m, ep):
        # tanh -> fp8 into half of a pair tile (PE v-dot reads pairs)
        nc.scalar.activation(
            th8[:, m % 2, :],
            ep[:],
            mybir.ActivationFunctionType.Tanh,
            bias=q_sb[:, m * BLOC + b : m * BLOC + b + 1],
            scale=DEQ,
        )

    def vdot_pe(th8_tiles):
        # score row via 4 DoubleRow matmuls; v sits in column 0 of a full
        # 128-wide stationary tile (DoubleRow needs >=16B between planes),
        # so row 0 of the psum carries the score and rows 1-127 are zero
        vp = psum_s.tile([128, NCHUNK], F32, tag="vs", name="vp")
        for j in range(4):
            nc.tensor.matmul(
                vp[:],
                lhsT=v8_sb[:, 2 * j : 2 * j + 2, :],
                rhs=th8_tiles[j][:],
                start=(j == 0),
                stop=(j == 3),
                perf_mode=DR,
            )
        return vp

    def flush_scores(vp, b, s):
        # DMA can't read PSUM: stage the score row through SBUF on DVE
        row = srow_pool.tile([1, NCHUNK], F32, name="srow")
        nc.vector.tensor_scalar_add(row[:], vp[0:1, :], 0.0)
        p16 = b * SCHUNKS + s
        nc.sync.dma_start(out=scores16[p16 : p16 + 1, :], in_=row[:])
        nc.sync.dma_start(
            out=scores4[b : b + 1, s * NCHUNK : (s + 1) * NCHUNK],
            in_=row[:],
        )
        if b == BLOC - 1:
            # all 4 rows of column-block s are final: exp this column strip
            # (hidden under the remaining chunks' PE work)
            nc.scalar.activation(
                expS[:, s * NCHUNK : (s + 1) * NCHUNK],
                scores4[:, s * NCHUNK : (s + 1) * NCHUNK],
                mybir.ActivationFunctionType.Exp,
                accum_out=esumq[:, s : s + 1],
            )

    # --- main loop: 16 chunks of 512 s-values ---------------------------
    # chunk i's v-dot matmuls are deferred into chunk i+1's PE stream
    # (after its 6th m-group) so the PE never waits on ScalarE tanh
    pend = None  # (th8_tiles, b, s) of chunk i-1
    for i in range(NCHUNKS):
        b, s = divmod(i, SCHUNKS)
        if i == 0:
            enc_sb = enc0_sb
        else:
            enc_sb = enc_pool.tile([128, KT, NCHUNK], F8, tag="enc")
            nc.sync.dma_start(
                out=enc_sb[:],
                in_=enc8_d[b].rearrange("k p s -> p k s")[
                    :, :, s * NCHUNK : (s + 1) * NCHUNK
                ],
            )
        if 4 <= i < 4 + MT:
            # refine weights: 2 MiB staggered per-m behind enc prefetch
            if i == 4:
                w16_sb = w_pool.tile([128, MT, KT * 128], F16, name="w16_sb")
            nc.sync.dma_start(out=w16_sb[:, i - 4, :], in_=w16_d[i - 4])
        th8_tiles = [
            tanh_pool.tile([128, 2, NCHUNK], F8, name="th8") for _ in range(4)
        ]
        m_start = 0
        if i == 0:
            # chunk 0: split-k for the first two m-groups so compute
            # starts once half of enc0 (+ first w8 block) has landed
            m_start = 2
            ep0 = psum_e.tile([128, NCHUNK], F32, tag="ep", name="ep")
            ep1 = psum_e.tile([128, NCHUNK], F32, tag="ep", name="ep")
            energy_part(ep0, enc_sb, 0, 0, 2)
            energy_part(ep1, enc_sb, 1, 0, 2)
            energy_part(ep0, enc_sb, 0, 2, KT // 2)
            tanh_m(th8_tiles[0], b, 0, ep0)
            energy_part(ep1, enc_sb, 1, 2, KT // 2)
            tanh_m(th8_tiles[0], b, 1, ep1)
        for m in range(m_start, MT):
            ep = energy_m(enc_sb, m, 0, KT // 2)
            if m == 6 and pend is not None:
                # deferred v-dot of chunk i-1 (its tanhs are long done)
                vp = vdot_pe(pend[0])
                flush_scores(vp, pend[1], pend[2])
                pend = None
            tanh_m(th8_tiles[m // 2], b, m, ep)
        pend = (th8_tiles, b, s)
        if i == NCHUNKS - 1:
            # speculative selection pass A: blocks 0..14 are final (only
            # chunk 15 pending) -- run the full selection and ship the
            # replicated index tile now; row 15's replicas get patched
            # after the last chunk.  Hides the DRAM round trip.
            nc.vector.max(vals16[:], scores16[:])
            idxA = score_pool.tile([16, NSEL], U16, name="idxA")
            nc.vector.max_index(idxA[:], vals16[:], scores16[:])
            nc.vector.tensor_scalar(
                sg16[:],
                idxA[:],
                blkoff_sb[0:16, 0:1],
                None,
                op0=mybir.AluOpType.bitwise_or,
            )
            nc.sync.dma_start(out=scr3_d, in_=sg16[:])
    # last chunk: v-dot immediately (PE waits on its final tanhs briefly)
    vp = vdot_pe(pend[0])
    flush_scores(vp, pend[1], pend[2])

    # --- selection pass B: final block-15 row ---------------------------
    nc.vector.max(vals16[:], scores16[:])
    idx16 = score_pool.tile([16, NSEL], U16, name="idx16")
    nc.vector.max_index(idx16[:], vals16[:], scores16[:])
    nc.vector.tensor_scalar(
        sg16[:],
        idx16[:],
        blkoff_sb[0:16, 0:1],
        None,
        op0=mybir.AluOpType.bitwise_or,
    )
    nc.sync.dma_start(out=outi_d[:], in_=sg16[:])
    # only block 15 changed vs pass A: patch its row in DRAM, then one
    # stride-0 broadcast read rebuilds all 8 replicas
    nc.sync.dma_start(out=scr3_d[15:16, :], in_=sg16[15:16, :])
    nc.sync.dma_start(
        out=sg128[:], in_=scr3_d.unsqueeze(0).broadcast_to((8, 16, NSEL))
    )
    # total exp-sum per row from the 4 column-strip accumulators
    nc.vector.tensor_reduce(
        esum4[:], esumq[:], axis=mybir.AxisListType.X, op=mybir.AluOpType.add
    )
    # selected approx scores to a [1,128] row in gather-column order
    # (via DRAM; latency hidden behind the gather + refine GEMM)
    valsd_d = aps["valsd_t"]  # [16, NSEL] f32 DRAM scratch
    nc.sync.dma_start(out=valsd_d, in_=vals16[:])
    vals_row = score_pool.tile([1, NIDX], F32, name="vals_row")
    nc.sync.dma_start(out=vals_row[:], in_=valsd_d.rearrange("p r -> r p"))
    expold_row = score_pool.tile([1, NIDX], F32, name="expold_row")
    nc.scalar.activation(
        expold_row[:], vals_row[:], mybir.ActivationFunctionType.Exp
    )

    # --- refine pass ----------------------------------------------------
    g_sb = score_pool.tile([128, KT, NIDX], F16, name="g_sb")
    nc.gpsimd.dma_gather(
        g_sb[:],
        enc16_d[:],
        sg128[:].bitcast(I16),
        num_idxs=NIDX,
        num_idxs_reg=NIDX,
        elem_size=HID,
        transpose=True,
    )

    # dummy matmuls during the selection/gather wait keep the PE clock
    # (HAM) warm so the refine GEMM runs at 2.4 GHz; the two leading
    # q-plane matmuls also slot in here (they don't need gathered data)
    epr01 = []
    for m in range(2):
        epr = psum_e.tile([128, NCHUNK], F32, tag="ep", name="epr")
        nc.tensor.matmul(
            epr[:, 0:NIDX],
            lhsT=qhl_sb[:, m * 128 : (m + 1) * 128],
            rhs=mask_sb[:],
            start=True,
            stop=False,
        )
        epr01.append(epr)
    wp2 = psum_s.tile([128, NCHUNK], F32, tag="vs", name="warm2")
    for _ in range(150):
        nc.tensor.matmul(
            wp2[:, 0:256],
            lhsT=scr[:, 0:128],
            rhs=scr[:, 0:256],
            start=True,
            stop=True,
        )
    vpr = psum_s.tile([1, NCHUNK], F32, tag="vs", name="vpr")
    thr_tiles = []
    for m in range(MT):
        if m < 2:
            epr = epr01[m]
            for k in range(KT):
                nc.tensor.matmul(
                    epr[:, 0:NIDX],
                    lhsT=w16_sb[:, m, k * 128 : (k + 1) * 128],
                    rhs=g_sb[:, k, :],
                    start=False,
                    stop=(k == KT - 1),
                )
        else:
            epr = psum_e.tile([128, NCHUNK], F32, tag="ep", name="epr")
            for k in range(KT):
                nc.tensor.matmul(
                    epr[:, 0:NIDX],
                    lhsT=w16_sb[:, m, k * 128 : (k + 1) * 128],
                    rhs=g_sb[:, k, :],
                    start=(k == 0),
                    stop=False,
                )
            nc.tensor.matmul(
                epr[:, 0:NIDX],
                lhsT=qhl_sb[:, m * 128 : (m + 1) * 128],
                rhs=mask_sb[:],
                start=False,
                stop=True,
            )
        if m >= 1:
            # deferred exact v-dot of group m-1 (its tanh is done)
            nc.tensor.matmul(
                vpr[0:1, 0:NIDX],
                lhsT=v16_sb[:, m - 1 : m],
                rhs=thr_tiles[m - 1][:],
                start=(m == 1),
                stop=False,
            )
        thr = thr_pool.tile([128, NIDX], F16, name="thr")
        nc.scalar.activation(
            thr[:],
            epr[:, 0:NIDX],
            mybir.ActivationFunctionType.Tanh,
            scale=1.0,
        )
        thr_tiles.append(thr)
    nc.tensor.matmul(
        vpr[0:1, 0:NIDX],
        lhsT=v16_sb[:, MT - 1 : MT],
        rhs=thr_tiles[MT - 1][:],
        start=False,
        stop=True,
    )

    # exp + delta stay in the [1,128] row domain; the per-row delta sums
    # come from a partition-broadcast read masked by a host one-hot, so no
    # DRAM hops sit on the critical path
    pk = score_pool.tile([1, 2 * NIDX], F32, name="pk")
    nc.scalar.activation(
        pk[:, 0:NIDX], vpr[0:1, 0:NIDX], mybir.ActivationFunctionType.Exp
    )
    nc.vector.scalar_tensor_tensor(
        pk[:, NIDX : 2 * NIDX],
        pk[:, 0:NIDX],
        1.0,
        expold_row[:],
        op0=mybir.AluOpType.mult,
        op1=mybir.AluOpType.subtract,
    )
    bc4 = score_pool.tile([BLOC, NIDX], F32, name="bc4")
    nc.gpsimd.partition_broadcast(
        bc4[:], pk[0:1, NIDX : 2 * NIDX], channels=BLOC
    )
    dzq4 = score_pool.tile([BLOC, NIDX], F32, name="dzq4")
    dz4 = score_pool.tile([BLOC, 1], F32, name="dz4")
    nc.vector.scalar_tensor_tensor(
        dzq4[:],
        bc4[:],
        1.0,
        bmask_sb[:],
        op0=mybir.AluOpType.mult,
        op1=mybir.AluOpType.mult,
        accum_out=dz4[:, 0:1],
    )
    z4 = score_pool.tile([BLOC, 1], F32, name="z4")
    nc.vector.scalar_tensor_tensor(
        z4[:],
        esum4[:],
        1.0,
        dz4[:],
        op0=mybir.AluOpType.mult,
        op1=mybir.AluOpType.add,
    )
    rcp4 = score_pool.tile([BLOC, 1], F32, name="rcp4")
    nc.vector.reciprocal(rcp4[:], z4[:])
    probs4 = score_pool.tile([BLOC, SRC], F32, name="probs4")
    half = SRC // 2
    nc.vector.tensor_scalar_mul(
        probs4[:, 0:half], expS[:, 0:half], rcp4[:, 0:1]
    )
    nc.sync.dma_start(out=outp_d[:, 0:half], in_=probs4[:, 0:half])
    nc.scalar.dma_start(out=outs_d[:], in_=pk[:, 0:NIDX])
    nc.scalar.dma_start(out=outz_d[:], in_=z4[:])
    nc.vector.tensor_scalar_mul(
        probs4[:, half:], expS[:, half:], rcp4[:, 0:1]
    )
    nc.scalar.dma_start(out=outp_d[:, half:], in_=probs4[:, half:])
    if "dbg_scores" in aps:
        nc.sync.dma_start(out=aps["dbg_scores"][:], in_=scores4[:])
        nc.sync.dma_start(out=aps["dbg_esum"][:], in_=esum4[:])
        nc.sync.dma_start(out=aps["dbg_dzq"][:], in_=dzq[:])
        nc.sync.dma_start(out=aps["dbg_g"][:], in_=g_sb[:])


def build_nc(dbg=False):
    nc = bacc.Bacc("TRN2", target_bir_lowering=False, debug=False)
    aps = {
        "enc8_t": nc.dram_tensor(
            "enc8_t", [BLOC, KT, 128, SRC], F8, kind="ExternalInput"
        ).ap(),
        "enc16_t": nc.dram_tensor(
            "enc16_t", [BLOC * SRC, HID], F16, kind="ExternalInput"
        ).ap(),
        "w8_t": nc.dram_tensor(
            "w8_t", [128, KT, MT * 128], F8, kind="ExternalInput"
        ).ap(),
        "w16_t": nc.dram_tensor(
            "w16_t", [MT, 128, KT * 128], F16, kind="ExternalInput"
        ).ap(),
        "q_t": nc.dram_tensor(
            "q_t", [128, MT * BLOC], F32, kind="ExternalInput"
        ).ap(),
        "qhl_t": nc.dram_tensor(
            "qhl_t", [8, MT * 128], F16, kind="ExternalInput"
        ).ap(),
        "mask_t": nc.dram_tensor(
            "mask_t", [8, NIDX], F16, kind="ExternalInput"
        ).ap(),
        "blkoff_t": nc.dram_tensor(
            "blkoff_t", [128, 1], U16, kind="ExternalInput"
        ).ap(),
        "bmask_t": nc.dram_tensor(
            "bmask_t", [BLOC, NIDX], F32, kind="ExternalInput"
        ).ap(),
        "v8_t": nc.dram_tensor(
            "v8_t", [128, KT, 128], F8, kind="ExternalInput"
        ).ap(),
        "v16_t": nc.dram_tensor(
            "v16_t", [128, MT], F16, kind="ExternalInput"
        ).ap(),
        "out_p": nc.dram_tensor(
            "out_p", [BLOC, SRC], F32, kind="ExternalOutput"
        ).ap(),
        "out_sel": nc.dram_tensor(
            "out_sel", [1, NIDX], F32, kind="ExternalOutput"
        ).ap(),
        "out_z": nc.dram_tensor(
            "out_z", [BLOC, 1], F32, kind="ExternalOutput"
        ).ap(),
        "out_idx": nc.dram_tensor(
            "out_idx", [16, NSEL], U16, kind="ExternalOutput"
        ).ap(),
        "scr_t": nc.dram_tensor(
            "scr_t", [16, 2 * NSEL], F32, kind="Internal"
        ).ap(),
        "valsd_t": nc.dram_tensor(
            "valsd_t", [16, NSEL], F32, kind="Internal"
        ).ap(),
        "scr2_t": nc.dram_tensor("scr2_t", [16], F32, kind="Internal").ap(),
        "scr3_t": nc.dram_tensor(
            "scr3_t", [16, NSEL], U16, kind="Internal"
        ).ap(),
    }
    if dbg:
        aps["dbg_scores"] = nc.dram_tensor(
            "dbg_scores", [BLOC, SRC], F32, kind="ExternalOutput"
        ).ap()
        aps["dbg_esum"] = nc.dram_tensor(
            "dbg_esum", [BLOC, 1], F32, kind="ExternalOutput"
        ).ap()
        aps["dbg_dzq"] = nc.dram_tensor(
            "dbg_dzq", [BLOC, 4], F32, kind="ExternalOutput"
        ).ap()
        aps["dbg_selex"] = nc.dram_tensor(
            "dbg_selex", [16, NSEL], F32, kind="ExternalOutput"
        ).ap()
        aps["dbg_expold"] = nc.dram_tensor(
            "dbg_expold", [16, NSEL], F32, kind="ExternalOutput"
        ).ap()
        aps["dbg_redr"] = nc.dram_tensor(
            "dbg_redr", [1, NIDX], F32, kind="ExternalOutput"
        ).ap()
        aps["dbg_g"] = nc.dram_tensor(
            "dbg_g", [128, KT, NIDX], F16, kind="ExternalOutput"
        ).ap()
    with tile.TileContext(nc, pool_alloc_mode="queue") as tc, ExitStack() as ctx:
        _build_kernel(ctx, tc, aps)
    nc.compile()
    return nc


def _prep_shared(hidden, attn_w, attn_b, v):
    w_e_t = np.ascontiguousarray(attn_w[:, HID:].T)  # [h, o]
    # fp8 weights: [h, o] -> [kt, 128, o] -> [128, kt, o], scaled x256
    w8_t = np.ascontiguousarray(
        (w_e_t * SW)
        .astype(ml_dtypes.float8_e4m3)
        .reshape(KT, 128, MT * 128)
        .transpose(1, 0, 2)
    )
    # fp16 refine weights (baseline layout)
    w16_t = np.ascontiguousarray(
        w_e_t.reshape(KT, 128, MT, 128)
        .transpose(2, 1, 0, 3)
        .reshape(MT, 128, KT * 128)
        .astype(np.float16)
    )
    v16_np = v.reshape(MT, 128).T  # [128, MT]
    v8_t = np.zeros((128, KT, 128), dtype=ml_dtypes.float8_e4m3)
    v8_t[:, :, 0] = v16_np.astype(ml_dtypes.float8_e4m3)
    v16_t = np.ascontiguousarray(v16_np.astype(np.float16))
    # q[o, b] = Wh @ hidden^T + b, fp32 on host (tiny GEMM)
    q_all = (hidden @ attn_w[:, :HID].T + attn_b).astype(np.float32)  # [B, H]
    q_cores = []
    qhl_cores = []
    for c in range(NCORES):
        qc = q_all[c * BLOC : (c + 1) * BLOC].T  # [HID, BLOC]
        q_cores.append(
            np.ascontiguousarray(
                qc.reshape(MT, 128, BLOC)
                .transpose(1, 0, 2)
                .reshape(128, MT * BLOC)
                .astype(np.float32)
            )
        )
        qb = q_all[c * BLOC : (c + 1) * BLOC]  # [BLOC, HID]
        qhi = qb.astype(np.float16)
        qlo = (qb - qhi.astype(np.float32)).astype(np.float16)
        qhl_cores.append(np.ascontiguousarray(np.concatenate([qhi, qlo], axis=0)))
    # one-hot mask: gathered column t belongs to b(t) = (t%16)//4
    mask = np.zeros((8, NIDX), dtype=np.float16)
    t = np.arange(NIDX)
    bt = (t % 16) // 4
    for j in range(4):
        mask[j, bt == j] = 1.0
        mask[j + 4, bt == j] = 1.0
    blkoff = ((np.arange(128, dtype=np.uint16) % 16) * 512).reshape(128, 1)
    t = np.arange(NIDX)
    bmask = ((t % 16) // 4 == np.arange(BLOC)[:, None]).astype(np.float32)
    return w8_t, w16_t, v8_t, v16_t, q_cores, qhl_cores, mask, blkoff, bmask


def kernel(hidden, encoder_outputs, attn_w, attn_b, v):
    global _last_results
    hidden = np.asarray(hidden, dtype=np.float32)
    encoder_outputs = np.asarray(encoder_outputs, dtype=np.float32)
    attn_w = np.asarray(attn_w, dtype=np.float32)
    attn_b = np.asarray(attn_b, dtype=np.float32)
    v = np.asarray(v, dtype=np.float32)

    if "nc" not in _compiled:
        _compiled["nc"] = build_nc()
    nc = _compiled["nc"]

    w8_t, w16_t, v8_t, v16_t, q_cores, qhl_cores, mask, blkoff, bmask = _prep_shared(
        hidden, attn_w, attn_b, v
    )
    in_maps = []
    for c in range(NCORES):
        enc_c = encoder_outputs[c * BLOC : (c + 1) * BLOC]  # [bloc, s, h]
        enc8_t = np.ascontiguousarray(
            (np.ascontiguousarray(enc_c.transpose(0, 2, 1)) * SE)
            .astype(ml_dtypes.float8_e4m3)
            .reshape(BLOC, KT, 128, SRC)
        )
        enc16_t = np.ascontiguousarray(
            enc_c.reshape(BLOC * SRC, HID).astype(np.float16)
        )
        in_maps.append(
            {
                "enc8_t": enc8_t,
                "enc16_t": enc16_t,
                "w8_t": w8_t,
                "w16_t": w16_t,
                "q_t": q_cores[c],
                "qhl_t": qhl_cores[c],
                "mask_t": mask,
                "blkoff_t": blkoff,
                "bmask_t": bmask,
                "v8_t": v8_t,
                "v16_t": v16_t,
            }
        )

    res = run_bass_kernel_spmd(nc, in_maps, list(range(NCORES)))
    _last_results = res
    out = np.empty((BATCH, SRC), dtype=np.float32)
    for c in range(NCORES):
        r = res.results[c]
        probs = np.asarray(r["out_p"], dtype=np.float32).copy()  # [BLOC, SRC]
        expsel = np.asarray(r["out_sel"], dtype=np.float32).ravel()  # [NIDX]
        z = np.asarray(r["out_z"], dtype=np.float32).reshape(BLOC)  # [BLOC]
        idxg = np.asarray(r["out_idx"]).astype(np.int64)  # [16, 8]
        t = np.arange(NIDX)
        idxr = idxg[t % 16, t // 16]  # gather-column order
        b_of = idxr // SRC
        pos = idxr % SRC
        probs[b_of, pos] = expsel / z[b_of]
        out[c * BLOC : (c + 1) * BLOC] = probs
    return out
